# revision 44
# baseline (speedup 1.0000x reference)
"""Trainium2 Bass kernel: single-head causal self-attention.

Problem: x[B=8, S=2048, D=1024], Wq/Wk/Wv[D, H=64], bq/bk/bv[H].
    q = x@Wq+bq; k = x@Wk+bk; v = x@Wv+bv
    out = softmax(causal(q k^T) / sqrt(H)) @ v

Sharding: batch -> 8 NeuronCores (data parallel, no collectives).

Per-core strategy:
  - x cast to bf16 on host, pre-transposed to xT [D, S]; each 512-seq
    chunk's col-halves stream on the two HWDGE queues (sync + scalar)
    concurrently (~330 GB/s aggregate), chunk 0 + a small first weight
    slice leading so the PE starts as early as possible
  - everything on the PE uses 2-way tile concurrency (the PE sustains
    ~1 col/cycle per tile at the observed mid p-state, two tiles run
    concurrently on disjoint row/col groups):
      * QK projection + the half-swap permutation matmul: col-tiled
        M=64 pairs (K^T -> psum partitions 0:64, Q^T -> 64:128)
      * V projection: col-tiled across chunk pairs (J even/odd)
      * scores: k-chunk pairs as two concurrent K=64 row-tiled matmuls
        (operands duplicated at both partition bases via the swap)
      * PV: two K=64 row-tiled halves accumulating into separate psum
        banks, summed on DVE at drain
  - causal trimming at 128-col granularity (scores/exp/PV); the 16
    diagonal [128,128] triangles masked multiplicatively on GpSimd
    after the ACT-engine exp (scale=1/8, bf16); softmax denominator
    rides as a ones-column in the PV stationary (memset)
  - O^T is PE-transposed back with a stride-4 column pick so each
    partition holds 4 consecutive output rows (1KB-contiguous store
    segments), normalized by the reciprocal of the ones column
  - single shared 6x1-bank PSUM ring + 2 PV accumulator banks; PV
    software-pipelined 3 chunks behind scores so exp latency hides
"""

import sys

sys.path.insert(0, "/opt/trn_rl_repo")

import numpy as np

B, S, D, H = 8, 2048, 1024, 64
N_CORES = 8
SQ = 512            # q chunk (PSUM bank / fp32 moving max)
NQ = S // SQ        # 4
ND = D // 128       # 8 contraction chunks for projections
H1 = H + 1          # V plus ones column

_CACHE = {}


def _build_nc(variant=()):
    variant = set(variant)
    import concourse.tile as tile
    from concourse import bacc, mybir

    f32 = mybir.dt.float32
    bf16 = mybir.dt.bfloat16
    AF = mybir.ActivationFunctionType

    nc = bacc.Bacc(None, target_bir_lowering=False)
    xTp = nc.dram_tensor("xTp", [NQ, 128, ND * SQ], bf16, kind="ExternalInput")
    wkq = nc.dram_tensor("wkq", [128, ND * 128], bf16, kind="ExternalInput")
    wv = nc.dram_tensor("wv", [128, ND * H], bf16, kind="ExternalInput")
    bkq = nc.dram_tensor("bkq", [128, 1], f32, kind="ExternalInput")
    bvv = nc.dram_tensor("bvv", [128, 1], f32, kind="ExternalInput")
    swp = nc.dram_tensor("swp", [128, 128], bf16, kind="ExternalInput")
    idt2 = nc.dram_tensor("idt2", [128, 128], bf16, kind="ExternalInput")
    idt65 = nc.dram_tensor("idt65", [H1, H1], bf16, kind="ExternalInput")
    maskT = nc.dram_tensor("maskT", [128, 128], bf16, kind="ExternalInput")
    out = nc.dram_tensor("out", [S, H], f32, kind="ExternalOutput")

    with tile.TileContext(nc) as tc:
        from contextlib import ExitStack

        with ExitStack() as ctx:
            const = ctx.enter_context(tc.tile_pool(name="const", bufs=1))
            sb = ctx.enter_context(tc.tile_pool(name="sb", bufs=1))
            pt_pool = ctx.enter_context(tc.tile_pool(name="pt", bufs=6))
            o_pool = ctx.enter_context(tc.tile_pool(name="o", bufs=2))
            ps = ctx.enter_context(tc.tile_pool(name="ps", bufs=1, space="PSUM"))

            # ---- constant + x loads ----
            # x B-halves ride the scalar HWDGE queue via just TWO issues
            # (each DIRECT2D costs ~1.4us of scalar-engine time, which
            # also runs the exps); everything else leads the sync queue
            # or the gpsimd SWDGE queue (small, late-needed consts).
            xt = {}      # J -> [128, ND*SQ] bf16 (c-chunk at cols c*SQ)
            QKT = {}     # J -> [128, SQ] bf16 (K^T rows 0:64, Q^T 64:128)
            KQ2 = {}     # J -> [128, SQ] bf16 (Q^T rows 0:64, K^T 64:128)
            Vones = {}   # t -> [128, 4*66] bf16 ((V|1) rows for k-chunks)

            HALF = ND * SQ // 2
            with nc.named_scope("load"):
                for J in range(NQ):
                    xt[J] = sb.tile([128, ND * SQ], bf16, tag=f"x{J}", name=f"xt{J}")

                # tiny first-chunk weights lead sync (ahead of x0A); the
                # rest of the weights lead the scalar queue ahead of x0B
                wkqB_sb = const.tile([128, 7 * 128], bf16, name="wkqB_sb")
                nc.scalar.dma_start(wkqB_sb[:], wkq[:, 128 : ND * 128])
                for J in range(NQ):
                    nc.scalar.dma_start(
                        xt[J][:, HALF : 2 * HALF], xTp[J, :, HALF : 2 * HALF]
                    )
                wkqA_sb = const.tile([128, 128], bf16, name="wkqA_sb")
                nc.sync.dma_start(wkqA_sb[:], wkq[:, 0:128])
                nc.sync.dma_start(xt[0][:, 0:HALF], xTp[0, :, 0:HALF])
                bkq_sb = const.tile([128, 1], f32)
                nc.sync.dma_start(bkq_sb[:], bkq[:, :])
                nc.sync.dma_start(xt[1][:, 0:HALF], xTp[1, :, 0:HALF])
                swp_sb = const.tile([128, 128], bf16)
                nc.sync.dma_start(swp_sb[:], swp[:, :])
                wv_sb = const.tile([128, ND * H], bf16)
                nc.sync.dma_start(wv_sb[:], wv[:, :])
                bvv_sb = const.tile([128, 1], f32)
                nc.sync.dma_start(bvv_sb[:], bvv[:, :])
                nc.sync.dma_start(xt[2][:, 0:HALF], xTp[2, :, 0:HALF])
                nc.sync.dma_start(xt[3][:, 0:HALF], xTp[3, :, 0:HALF])
                maskT_sb = const.tile([128, 128], bf16)
                nc.gpsimd.dma_start(maskT_sb[:], maskT[:, :])
                idt2_sb = const.tile([128, 128], bf16)
                nc.gpsimd.dma_start(idt2_sb[:], idt2[:, :])
                idt65_sb = const.tile([H1, H1], bf16)
                nc.gpsimd.dma_start(idt65_sb[:], idt65[:, :])

            SWQ = {}

            def proj_qk(J):
                # col-tiled pair: K^T to psum partitions 0:64, Q^T to
                # 64:128 — two concurrent M=64 matmuls on distinct
                # col-groups double the effective stream rate
                with nc.named_scope(f"proj{J}"):
                    qs = ps.tile([128, SQ], f32, tag="ps", bufs=6)
                    for c in range(ND):
                        w_sb = wkqA_sb if c == 0 else wkqB_sb
                        co = 0 if c == 0 else (c - 1) * 128
                        nc.tensor.matmul(
                            qs[0:H, :],
                            w_sb[:, co : co + H],
                            xt[J][:, c * SQ : (c + 1) * SQ],
                            start=(c == 0),
                            stop=(c == ND - 1),
                            skip_group_check=True,
                        )
                        nc.tensor.matmul(
                            qs[H : 2 * H, :],
                            w_sb[:, co + H : co + 128],
                            xt[J][:, c * SQ : (c + 1) * SQ],
                            start=(c == 0),
                            stop=(c == ND - 1),
                            skip_group_check=True,
                        )
                    qkt = sb.tile([128, SQ], bf16, tag=f"qkt{J}")
                    nc.vector.tensor_scalar_add(qkt[:], qs[:], bkq_sb[:])
                    QKT[J] = qkt
                    SWQ[J] = qs

            def swap_qk(J):
                # kq2 = [[0,I],[I,0]] @ qkt: Q^T at base 0, K^T at 64;
                # col-tiled halves run concurrently
                with nc.named_scope(f"swap{J}"):
                    SWQ.pop(J)
                    sw = ps.tile([128, SQ], f32, tag="ps", bufs=6)
                    nc.tensor.matmul(
                        sw[0:H, :], swp_sb[:, 0:H], QKT[J][:],
                        start=True, stop=True, skip_group_check=True,
                    )
                    nc.tensor.matmul(
                        sw[H : 2 * H, :], swp_sb[:, H:128], QKT[J][:],
                        start=True, stop=True, skip_group_check=True,
                    )
                    kq2 = sb.tile([128, SQ], bf16, tag=f"kq2{J}")
                    nc.vector.tensor_copy(kq2[:], sw[:])
                    KQ2[J] = kq2

            def proj_v(J0, J1):
                # col-tiled pair: V^T of J0 at psum partitions 0:64, J1 at
                # 64:128, in one moving stream per contraction chunk
                with nc.named_scope(f"projv{J0}"):
                    vs = ps.tile([128, SQ], f32, tag="ps", bufs=6)
                    for c in range(ND):
                        # col-tiled pair shares one psum bank on disjoint
                        # partition slices; the sim's group check is
                        # bank-granular, so bypass it
                        nc.tensor.matmul(
                            vs[0:H, 0:SQ],
                            wv_sb[:, c * H : (c + 1) * H],
                            xt[J0][:, c * SQ : (c + 1) * SQ],
                            start=(c == 0),
                            stop=(c == ND - 1),
                            skip_group_check=True,
                        )
                        nc.tensor.matmul(
                            vs[H : 2 * H, 0:SQ],
                            wv_sb[:, c * H : (c + 1) * H],
                            xt[J1][:, c * SQ : (c + 1) * SQ],
                            start=(c == 0),
                            stop=(c == ND - 1),
                            skip_group_check=True,
                        )
                    vt2 = sb.tile([128, SQ], bf16, tag=f"vt2{J0}")
                    # half-drains so the first transposes start sooner
                    nc.vector.tensor_scalar_add(
                        vt2[:, 0:256], vs[:, 0:256], bvv_sb[:]
                    )
                    nc.vector.tensor_scalar_add(
                        vt2[:, 256:SQ], vs[:, 256:SQ], bvv_sb[:]
                    )
                    # transpose full [128,128] blocks: each output block has
                    # J0's V rows in cols 0:64 and J1's in cols 64:128
                    pvt = ps.tile([128, 8 * H], bf16, tag="ps", bufs=6)
                    for tt in range(4):
                        nc.tensor.transpose(
                            pvt[:, tt * 128 : (tt + 1) * 128],
                            vt2[:, tt * 128 : (tt + 1) * 128],
                            idt2_sb[:, :],
                        )
                    for j, J in ((0, J0), (1, J1)):
                        vo = sb.tile([128, 4 * 66], bf16, tag=f"vo{J}")
                        # ones columns: fill whole tile, data copy overwrites
                        nc.vector.memset(vo[:], 1.0)
                        nc.vector.tensor_copy(
                            vo[:].rearrange("p (t u) -> p t u", t=4)[:, :, 0:H],
                            pvt[:].rearrange("p (t u) -> p t u", t=4)[
                                :, :, j * H : (j + 1) * H
                            ],
                        )
                        Vones[J] = vo

            def att(J):
                with nc.named_scope(f"att{J}"):
                    # PV split into two K=64 row-tiled halves accumulating
                    # into separate banks (summed at drain) — concurrent
                    # row groups double the effective stream rate
                    ota = ps.tile([H1, SQ], f32, tag="ota", bufs=1)
                    otb = ps.tile([H1, SQ], f32, tag="otb", bufs=1)
                    nch = 4 * (J + 1)
                    LAG = 3  # chunks of PV lag behind scores
                    pts = []  # (pt tile, lo) pending PV per chunk
                    for m in range(nch + LAG):
                        if m < nch and m % 2 == 0:
                            # row-tiled scores pair for chunks m, m+1
                            for i in (m, m + 1):
                                lo = max(0, (i - 4 * J) * 128)
                                st = ps.tile(
                                    [128, SQ], f32, tag="ps", bufs=6,
                                    name=f"st{J}_{i}",
                                )
                                if i % 2 == 0:
                                    nc.tensor.matmul(
                                        st[:, lo:SQ],
                                        QKT[i // 4][0:H, (i % 4) * 128 : (i % 4 + 1) * 128],
                                        KQ2[J][0:H, lo:SQ],
                                        start=True, stop=True,
                                    )
                                else:
                                    nc.tensor.matmul(
                                        st[:, lo:SQ],
                                        KQ2[i // 4][H : 2 * H, (i % 4) * 128 : (i % 4 + 1) * 128],
                                        QKT[J][H : 2 * H, lo:SQ],
                                        start=True, stop=True,
                                    )
                                pt = pt_pool.tile(
                                    [128, SQ], bf16, tag="pt", bufs=8,
                                    name=f"pt{J}_{i}",
                                )
                                nc.scalar.activation(
                                    pt[:, lo:SQ], st[:, lo:SQ],
                                    AF.Exp, scale=0.125,
                                )
                                if i >= 4 * J:  # diagonal: mask the triangle
                                    nc.gpsimd.tensor_mul(
                                        pt[:, lo : lo + 128],
                                        pt[:, lo : lo + 128],
                                        maskT_sb[:],
                                    )
                                pts.append((pt, lo))
                        if m >= LAG:  # PV of chunk m-LAG (software-pipelined)
                            i = m - LAG
                            pt, lo = pts[i]
                            vo_s = Vones[i // 4][
                                :, (i % 4) * 66 : (i % 4) * 66 + H1
                            ]
                            nc.tensor.matmul(
                                ota[:, lo:SQ],
                                vo_s[0:H, :],
                                pt[0:H, lo:SQ],
                                start=(i == 0),
                                stop=(i == nch - 1),
                            )
                            nc.tensor.matmul(
                                otb[:, lo:SQ],
                                vo_s[H:128, :],
                                pt[H:128, lo:SQ],
                                start=(i == 0),
                                stop=(i == nch - 1),
                            )
                    return ota, otb

            def store(J, ot):
                ota, otb = ot
                with nc.named_scope(f"out{J}"):
                    # DVE has a single PSUM port: drain otb to SBUF, then add
                    obs = o_pool.tile([H1, SQ], f32, tag="obs")
                    nc.vector.tensor_copy(obs[:], otb[:])
                    ots = o_pool.tile([H1, SQ], bf16, tag="ots")
                    nc.vector.tensor_add(ots[:], ota[:], obs[:])
                    po = ps.tile([128, 4 * 66], bf16, tag="ps", bufs=6)
                    # stride-4 column pick: block tt holds q rows 4p+tt, so
                    # each partition stores 4 consecutive output rows and
                    # the store becomes 1KB-contiguous DMA segments
                    for tt in range(4):
                        nc.tensor.transpose(
                            po[:, tt * 66 : tt * 66 + H1],
                            ots[:, tt : tt + 509 : 4],
                            idt65_sb[:, :],
                        )
                    rc = o_pool.tile([128, 4], f32, tag="rc")
                    nc.vector.reciprocal(rc[:], po[:, H :: 66])
                    ob = o_pool.tile([128, 4 * H], f32, tag="ob")
                    for tt in range(4):
                        nc.vector.tensor_scalar_mul(
                            ob[:, tt * H : (tt + 1) * H],
                            po[:, tt * 66 : tt * 66 + H],
                            rc[:, tt : tt + 1],
                        )
                    nc.sync.dma_start(
                        out[J * SQ : (J + 1) * SQ, :].rearrange(
                            "(p t) h -> p t h", p=128
                        ),
                        ob[:].rearrange("p (t h) -> p t h", t=4),
                    )

            # stores are emitted one att late so the PE keeps streaming the
            # next att's scores while DVE drains the finished ot
            proj_qk(0)
            proj_qk(1)
            swap_qk(0)
            swap_qk(1)
            proj_v(0, 1)
            ot0 = att(0)
            ot1 = att(1)
            store(0, ot0)
            proj_qk(2)
            proj_qk(3)
            swap_qk(2)
            swap_qk(3)
            store(1, ot1)
            proj_v(2, 3)
            ot2 = att(2)
            ot3 = att(3)
            store(2, ot2)
            store(3, ot3)

    nc.finalize()
    return nc


def _host_prep(x, Wq, bq, Wk, bk, Wv, bv):
    """Layout-only host prep: shard x by batch + pack weight operands."""
    import ml_dtypes

    f32 = np.float32
    bf = ml_dtypes.bfloat16
    wkq = np.concatenate([Wk, Wq], axis=1)          # [D, 128]
    wkq = np.ascontiguousarray(
        wkq.reshape(ND, 128, 128).transpose(1, 0, 2).reshape(128, ND * 128)
    ).astype(bf)
    wv = np.ascontiguousarray(
        Wv.reshape(ND, 128, H).transpose(1, 0, 2).reshape(128, ND * H)
    ).astype(bf)
    bkq = np.ascontiguousarray(np.concatenate([bk, bq])[:, None], dtype=f32)
    bvv = np.ascontiguousarray(np.concatenate([bv, bv])[:, None], dtype=f32)
    i64 = np.eye(64, dtype=f32)
    z64 = np.zeros((64, 64), dtype=f32)
    swp = np.block([[z64, i64], [i64, z64]]).astype(bf)
    idt2 = np.eye(128, dtype=f32).astype(bf)
    idt65 = np.eye(H1, dtype=f32).astype(bf)
    kk = np.arange(128)[:, None]
    cc = np.arange(128)[None, :]
    maskT = (cc >= kk).astype(bf)
    common = {
        "wkq": wkq, "wv": wv, "bkq": bkq, "bvv": bvv,
        "swp": swp, "idt2": idt2, "idt65": idt65, "maskT": maskT,
    }
    in_maps = []
    for b in range(B):
        m = dict(common)
        # xTp[J, p, c*SQ+s] = x[b][SQ*J+s, 128*c+p]
        m["xTp"] = np.ascontiguousarray(
            x[b].reshape(NQ, SQ, ND, 128).transpose(0, 3, 2, 1)
        ).astype(bf).reshape(NQ, 128, ND * SQ)
        in_maps.append(m)
    return in_maps


def run(x, Wq, bq, Wk, bk, Wv, bv, trace=False):
    from concourse.bass_utils import run_bass_kernel_spmd

    if "nc" not in _CACHE:
        _CACHE["nc"] = _build_nc()
    nc = _CACHE["nc"]
    in_maps = _host_prep(
        np.asarray(x), np.asarray(Wq), np.asarray(bq), np.asarray(Wk),
        np.asarray(bk), np.asarray(Wv), np.asarray(bv),
    )
    res = run_bass_kernel_spmd(
        nc, in_maps, core_ids=list(range(N_CORES)), trace=trace
    )
    outs = np.stack([res.results[c]["out"] for c in range(N_CORES)], axis=0)
    return outs.astype(np.float32), res


def kernel(x, Wq, bq, Wk, bk, Wv, bv):
    outs, _ = run(x, Wq, bq, Wk, bk, Wv, bv, trace=False)
    return outs
